# revision 10
# baseline (speedup 1.0000x reference)
"""Trainium2 Bass kernel for the Attention3 module (B=128, S=1024, RNN=2048, HID=512).

Strategy: data-parallel over batch B across 8 NeuronCores (16 batches/core).
Host side only reshapes/transposes inputs into DMA-friendly layouts; all model
compute (MLP, tanh, scores, softmax, weighted sum) runs on device.

Per-core device pipeline:
  1. MLP: att_h = h@W1.T+b1 @W2.T+b2 @W3.T+b3 @W4.T+b4   (PE, float32r)
     - activations kept transposed ([K,16] lhsT tiles); weights pre-transposed
       on host so the contraction dim lands on partitions.
     - biases folded in as K=1 ones-outer-product matmuls into the same PSUM
       accumulation group.
  2. scores: tanh(p_att^T + att_h) with HID on partitions, so the att_h add is
     a fused per-partition bias on ScalarE (in-place on the streamed p tile);
     Wa contraction is a PE matmul whose stationary operand column m holds Wa
     masked to batch b (zero elsewhere), so every batch accumulates into its
     own PSUM row of one shared [16, 512] accumulation group per s-half.
     Mask+ba applied as a precomputed additive term during PSUM evacuation.
  3. softmax over S on [16, 1024] (free-dim reductions + fused exp bias).
  4. weighted sum: PE-transpose softmax weights into a block-diagonal
     [S, b, m] masked layout, then stream att_feats tiles [128, 2, 2048] and
     matmul (float32r); each batch lands in its own row of shared [16, 512]
     PSUM groups.

DMA: issued on both HWDGE rings (nc.sync = SP, nc.scalar = ACT) to keep two
transfers in flight; all bulk transfers are >= 2 MiB.
"""

import functools

import numpy as np

import concourse.bacc as bacc
import concourse.bass as bass
import concourse.tile as tile
from concourse import mybir
from concourse.bass_utils import run_bass_kernel_spmd
from concourse.masks import make_identity

N_CORES = 8
B, S, RNN, HID = 128, 1024, 2048, 512
BPC = B // N_CORES  # batches per core
F32 = mybir.dt.float32
F32R = mybir.dt.float32r
MASK_NEG = -1.0e9
AX_X = mybir.AxisListType.X
TANH = mybir.ActivationFunctionType.Tanh
EXP = mybir.ActivationFunctionType.Exp

NHT = HID // 128  # 4 h-tiles
NST = S // 128  # 8 s-tiles
FU = 2  # s-tiles per att_feats DMA


def _build_body(ctx, tc, io):
    nc = tc.nc

    consts = ctx.enter_context(tc.tile_pool(name="consts", bufs=1))
    wpool = ctx.enter_context(tc.tile_pool(name="wpool", bufs=3))
    mlp = ctx.enter_context(tc.tile_pool(name="mlp", bufs=1))
    ppool = ctx.enter_context(tc.tile_pool(name="ppool", bufs=3))
    fpool = ctx.enter_context(tc.tile_pool(name="fpool", bufs=4))
    psA = ctx.enter_context(tc.tile_pool(name="psA", bufs=3, space="PSUM"))
    psB = ctx.enter_context(tc.tile_pool(name="psB", bufs=4, space="PSUM"))

    # ---- constants / small inputs ----
    ident = consts.tile([128, 128], F32)
    make_identity(nc, ident)
    ones_f = consts.tile([1, BPC], F32)
    nc.vector.memset(ones_f, 1.0)
    ones1 = consts.tile([1, BPC], F32R)
    nc.vector.tensor_copy(out=ones1, in_=ones_f)

    bias_sb = []
    for i, o in enumerate([1024, 1024, 512, 512]):
        t = consts.tile([1, o], F32R, tag=f"b{i + 1}")
        nc.sync.dma_start(out=t, in_=io[f"b{i + 1}"])
        bias_sb.append(t)

    wa_sb = consts.tile([128, NHT * BPC * BPC], F32R)
    nc.sync.dma_start(out=wa_sb, in_=io["warep"])
    wa_m = wa_sb.rearrange("p (t b m) -> p t b m", t=NHT, b=BPC)

    madd_sb = consts.tile([BPC, S], F32)
    nc.sync.dma_start(out=madd_sb, in_=io["madd"])

    hT_sb = consts.tile([128, RNN // 128, BPC], F32R)
    nc.sync.dma_start(
        out=hT_sb, in_=io["hT"].rearrange("(u p) b -> p u b", p=128)
    )

    # ---- phase 1: MLP (float32r matmuls) ----
    def layer(xT_sb, K, O, wt_dram, bias_t, name):
        y_sb = mlp.tile([BPC, O], F32, tag=f"y_{name}")
        nch = O // 512
        pss = [
            psA.tile([BPC, 512], F32, tag="ps_small", name=f"ps_y{name}_{n}")
            for n in range(nch)
        ]
        for n in range(nch):
            nc.tensor.matmul(
                pss[n],
                lhsT=ones1,
                rhs=bias_t[0:1, n * 512 : (n + 1) * 512],
                start=True,
                stop=False,
            )
        kt = K // 128
        for k in range(kt):
            wt = wpool.tile([128, O], F32R, tag="wt")
            nc.sync.dma_start(out=wt, in_=wt_dram[k * 128 : (k + 1) * 128, :])
            for n in range(nch):
                nc.tensor.matmul(
                    pss[n],
                    lhsT=xT_sb[:, k, :],
                    rhs=wt[:, n * 512 : (n + 1) * 512],
                    start=False,
                    stop=(k == kt - 1),
                )
        for n in range(nch):
            nc.scalar.copy(out=y_sb[:, n * 512 : (n + 1) * 512], in_=pss[n])
        return y_sb

    def transpose_rows(y_sb, O, name, dtype=F32R):
        yT = mlp.tile([128, O // 128, BPC], dtype, tag=f"yT_{name}")
        for j in range(O // 128):
            ps = psA.tile([128, BPC], F32, tag="ps_small")
            nc.tensor.transpose(ps, y_sb[:, j * 128 : (j + 1) * 128], ident[:BPC, :BPC])
            nc.vector.tensor_copy(out=yT[:, j, :], in_=ps)
        return yT

    y1 = layer(hT_sb, RNN, 1024, io["w1t"], bias_sb[0], "1")
    y1T = transpose_rows(y1, 1024, "1")
    y2 = layer(y1T, 1024, 1024, io["w2t"], bias_sb[1], "2")
    y2T = transpose_rows(y2, 1024, "2")
    y3 = layer(y2T, 1024, 512, io["w3t"], bias_sb[2], "3")
    y3T = transpose_rows(y3, 512, "3")
    ah = layer(y3T, 512, 512, io["w4t"], bias_sb[3], "4")
    ahT = transpose_rows(ah, 512, "ah", dtype=F32)  # [128, NHT, BPC]

    # ---- phase 2: scores = Wa . tanh(p^T + att_h) + (mask_add + ba) ----
    # lhsT column m of wa_m[:, ht, b, :] is Wa if m == b else 0, so batch b's
    # matmuls only contribute to PSUM row b; all 16 batches (x NHT k-tiles)
    # accumulate into one [BPC, 512] PSUM group per s-half.
    scores = mlp.tile([BPC, S], F32, tag="scores")
    nsh = S // 512
    ps_sc = [
        psA.tile([BPC, 512], F32, tag="ps_small", name=f"ps_sc_{sh}")
        for sh in range(nsh)
    ]
    for b in range(BPC):
        pt = ppool.tile([128, NHT, S], F32R, tag="pt")
        nc.scalar.dma_start(
            out=pt, in_=io["pT"][b].rearrange("(u p) s -> p u s", p=128)
        )
        for ht in range(NHT):
            nc.scalar.activation(
                out=pt[:, ht, :],
                in_=pt[:, ht, :],
                func=TANH,
                bias=ahT[:, ht, b : b + 1],
                scale=1.0,
            )
        for sh in range(nsh):
            for ht in range(NHT):
                nc.tensor.matmul(
                    ps_sc[sh],
                    lhsT=wa_m[:, ht, b, :],
                    rhs=pt[:, ht, sh * 512 : (sh + 1) * 512],
                    start=(b == 0 and ht == 0),
                    stop=(b == BPC - 1 and ht == NHT - 1),
                )
    for sh in range(nsh):
        nc.vector.tensor_add(
            out=scores[:, sh * 512 : (sh + 1) * 512],
            in0=ps_sc[sh],
            in1=madd_sb[:, sh * 512 : (sh + 1) * 512],
        )

    # ---- phase 2.5: softmax over S (in place on scores) ----
    mx = mlp.tile([BPC, 1], F32, tag="mx")
    nc.vector.reduce_max(out=mx, in_=scores, axis=AX_X)
    nmx = mlp.tile([BPC, 1], F32, tag="nmx")
    nc.vector.tensor_scalar_mul(out=nmx, in0=mx, scalar1=-1.0)
    ssum = mlp.tile([BPC, 1], F32, tag="ssum")
    nc.scalar.activation(
        out=scores, in_=scores, func=EXP, bias=nmx, scale=1.0, accum_out=ssum
    )
    rs = mlp.tile([BPC, 1], F32, tag="rs")
    nc.vector.reciprocal(out=rs, in_=ssum)
    nc.vector.tensor_scalar_mul(out=scores, in0=scores, scalar1=rs)

    # Block-diagonal masked weights: w_mask[:, t, b, m] = w[s, b] if m == b
    # else 0, so batch b's matvec only writes PSUM row b.  Zeroed via a cast
    # copy from an f32 scratch (memset can't encode float32r), then the
    # PE-transposed softmax weights are written straight onto the diagonal.
    w_mask = mlp.tile([128, NST, BPC, BPC], F32R, tag="w_mask")
    zsrc = mlp.tile([128, NST * BPC * BPC], F32, tag="zsrc")
    nc.vector.memset(zsrc, 0.0)
    nc.vector.tensor_copy(out=w_mask.rearrange("p a b c -> p (a b c)"), in_=zsrc)
    for t in range(NST):
        ps = psA.tile([128, BPC], F32, tag="ps_small")
        nc.tensor.transpose(ps, scores[:, t * 128 : (t + 1) * 128], ident[:BPC, :BPC])
        sl = w_mask[:, t, :, :]
        diag_ap = bass.AP(
            tensor=sl.tensor,
            offset=sl.offset,
            ap=[sl.ap[0], [sl.ap[1][0] + sl.ap[2][0], BPC]],
        )
        nc.vector.tensor_copy(out=diag_ap, in_=ps)

    # ---- phase 3: att_res[b] = sum_s w[b,s] * att_feats[b,s,:] ----
    out_sb = mlp.tile([BPC, RNN], F32, tag="out_sb")
    nn = RNN // 512
    ps_mv = [
        psB.tile([BPC, 512], F32, tag="mv", name=f"ps_mv_{n}") for n in range(nn)
    ]
    for b in range(BPC):
        for tc_i in range(NST // FU):
            ft = fpool.tile([128, FU, RNN], F32R, tag="ft")
            eng = nc.sync if (b * (NST // FU) + tc_i) % 2 == 0 else nc.scalar
            eng.dma_start(
                out=ft,
                in_=io["f"][b, tc_i * FU * 128 : (tc_i + 1) * FU * 128, :].rearrange(
                    "(u p) d -> p u d", p=128
                ),
            )
            for u in range(FU):
                t = tc_i * FU + u
                for n in range(nn):
                    nc.tensor.matmul(
                        ps_mv[n],
                        lhsT=w_mask[:, t, b, :],
                        rhs=ft[:, u, n * 512 : (n + 1) * 512],
                        start=(b == 0 and t == 0),
                        stop=(b == BPC - 1 and t == NST - 1),
                    )
    for n in range(nn):
        nc.vector.tensor_copy(out=out_sb[:, n * 512 : (n + 1) * 512], in_=ps_mv[n])
    nc.sync.dma_start(out=io["out"], in_=out_sb)


def _build():
    from contextlib import ExitStack

    nc = bacc.Bacc("TRN2", target_bir_lowering=False, debug=False, num_devices=N_CORES)
    io = {
        "hT": nc.dram_tensor("hT", [RNN, BPC], F32R, kind="ExternalInput").ap(),
        "pT": nc.dram_tensor("pT", [BPC, HID, S], F32R, kind="ExternalInput").ap(),
        "f": nc.dram_tensor("f", [BPC, S, RNN], F32R, kind="ExternalInput").ap(),
        "madd": nc.dram_tensor("madd", [BPC, S], F32, kind="ExternalInput").ap(),
        "w1t": nc.dram_tensor("w1t", [RNN, 1024], F32R, kind="ExternalInput").ap(),
        "w2t": nc.dram_tensor("w2t", [1024, 1024], F32R, kind="ExternalInput").ap(),
        "w3t": nc.dram_tensor("w3t", [1024, 512], F32R, kind="ExternalInput").ap(),
        "w4t": nc.dram_tensor("w4t", [512, 512], F32R, kind="ExternalInput").ap(),
        "b1": nc.dram_tensor("b1", [1, 1024], F32R, kind="ExternalInput").ap(),
        "b2": nc.dram_tensor("b2", [1, 1024], F32R, kind="ExternalInput").ap(),
        "b3": nc.dram_tensor("b3", [1, 512], F32R, kind="ExternalInput").ap(),
        "b4": nc.dram_tensor("b4", [1, 512], F32R, kind="ExternalInput").ap(),
        "warep": nc.dram_tensor(
            "warep", [128, NHT * BPC * BPC], F32R, kind="ExternalInput"
        ).ap(),
        "out": nc.dram_tensor("out", [BPC, RNN], F32, kind="ExternalOutput").ap(),
    }
    with tile.TileContext(nc) as tc:
        with ExitStack() as ctx:
            _build_body(ctx, tc, io)
    nc.compile()
    return nc


@functools.lru_cache(maxsize=1)
def _get_nc():
    return _build()


def _prep_in_maps(h, att_feats, p_att_feats, mask, W1, b1, W2, b2, W3, b3, W4, b4, Wa, ba):
    f32 = np.float32
    asc = np.ascontiguousarray

    def a(x):
        return np.asarray(x, dtype=f32)

    w1t = asc(a(W1).T)
    w2t = asc(a(W2).T)
    w3t = asc(a(W3).T)
    w4t = asc(a(W4).T)
    b1r = a(b1).reshape(1, -1)
    b2r = a(b2).reshape(1, -1)
    b3r = a(b3).reshape(1, -1)
    b4r = a(b4).reshape(1, -1)
    wa = a(Wa).reshape(-1)  # [HID]
    warep = np.zeros((128, NHT, BPC, BPC), dtype=f32)
    for ht in range(NHT):
        for b in range(BPC):
            warep[:, ht, b, b] = wa[ht * 128 : (ht + 1) * 128]
    warep = warep.reshape(128, NHT * BPC * BPC)
    ba0 = float(np.asarray(ba).reshape(-1)[0])

    h = a(h)
    p = a(p_att_feats)
    f = np.asarray(att_feats, dtype=f32)
    m = np.asarray(mask)

    in_maps = []
    for c in range(N_CORES):
        sl = slice(c * BPC, (c + 1) * BPC)
        in_maps.append(
            {
                "hT": asc(h[sl].T),
                "pT": asc(p[sl].transpose(0, 2, 1)),
                "f": asc(f[sl]),
                "madd": (m[sl].astype(f32) * MASK_NEG + ba0).astype(f32),
                "w1t": w1t,
                "w2t": w2t,
                "w3t": w3t,
                "w4t": w4t,
                "b1": b1r,
                "b2": b2r,
                "b3": b3r,
                "b4": b4r,
                "warep": warep,
            }
        )
    return in_maps


def _run(in_maps, trace=False):
    nc = _get_nc()
    res = run_bass_kernel_spmd(nc, in_maps, core_ids=list(range(N_CORES)), trace=trace)
    out = np.concatenate([res.results[c]["out"] for c in range(N_CORES)], axis=0)
    return out, res


def kernel(h, att_feats, p_att_feats, mask, W1, b1, W2, b2, W3, b3, W4, b4, Wa, ba):
    in_maps = _prep_in_maps(
        h, att_feats, p_att_feats, mask, W1, b1, W2, b2, W3, b3, W4, b4, Wa, ba
    )
    out, _ = _run(in_maps)
    return out


# revision 11
# speedup vs baseline: 1.7044x; 1.7044x over previous
"""Trainium2 Bass kernel for the Attention3 module (B=128, S=1024, RNN=2048, HID=512).

Strategy: data-parallel over batch B across 8 NeuronCores (16 batches/core).
Host side only reshapes/transposes inputs into DMA-friendly layouts; all model
compute (MLP, tanh, scores, softmax, weighted sum) runs on device.

Per-core device pipeline:
  1. MLP: att_h = h@W1.T+b1 @W2.T+b2 @W3.T+b3 @W4.T+b4   (PE, float32r)
     - activations kept transposed ([K,16] lhsT tiles); weights pre-transposed
       on host so the contraction dim lands on partitions.
     - biases folded in as K=1 ones-outer-product matmuls into the same PSUM
       accumulation group.
  2. scores: tanh(p_att^T + att_h) with HID on partitions, so the att_h add is
     a fused per-partition bias on ScalarE (in-place on the streamed p tile);
     Wa contraction is a PE matmul whose stationary operand column m holds Wa
     masked to batch b (zero elsewhere), so every batch accumulates into its
     own PSUM row of one shared [16, 512] accumulation group per s-half.
     Mask+ba applied as a precomputed additive term during PSUM evacuation.
  3. softmax over S on [16, 1024] (free-dim reductions + fused exp bias).
  4. weighted sum: PE-transpose softmax weights into a block-diagonal
     [S, b, m] masked layout, then stream att_feats tiles [128, 2, 2048] and
     matmul (float32r); each batch lands in its own row of shared [16, 512]
     PSUM groups.

DMA: issued on both HWDGE rings (nc.sync = SP, nc.scalar = ACT) to keep two
transfers in flight; all bulk transfers are >= 2 MiB.
"""

import functools

import ml_dtypes
import numpy as np

import concourse.bacc as bacc
import concourse.bass as bass
import concourse.tile as tile
from concourse import mybir
from concourse.bass_utils import run_bass_kernel_spmd
from concourse.masks import make_identity

N_CORES = 8
B, S, RNN, HID = 128, 1024, 2048, 512
BPC = B // N_CORES  # batches per core
F32 = mybir.dt.float32
F32R = mybir.dt.float32r
MASK_NEG = -1.0e9
AX_X = mybir.AxisListType.X
TANH = mybir.ActivationFunctionType.Tanh
EXP = mybir.ActivationFunctionType.Exp
BF16 = mybir.dt.bfloat16

NHT = HID // 128  # 4 h-tiles
NST = S // 128  # 8 s-tiles
FU = 2  # s-tiles per att_feats DMA


def _build_body(ctx, tc, io):
    nc = tc.nc

    consts = ctx.enter_context(tc.tile_pool(name="consts", bufs=1))
    wpool = ctx.enter_context(tc.tile_pool(name="wpool", bufs=3))
    mlp = ctx.enter_context(tc.tile_pool(name="mlp", bufs=1))
    ppool = ctx.enter_context(tc.tile_pool(name="ppool", bufs=4))
    fpool = ctx.enter_context(tc.tile_pool(name="fpool", bufs=10))
    psA = ctx.enter_context(tc.tile_pool(name="psA", bufs=3, space="PSUM"))
    psB = ctx.enter_context(tc.tile_pool(name="psB", bufs=4, space="PSUM"))

    # ---- constants / small inputs ----
    ident = consts.tile([128, 128], F32)
    make_identity(nc, ident)
    ones_f = consts.tile([1, BPC], F32)
    nc.vector.memset(ones_f, 1.0)
    ones1 = consts.tile([1, BPC], F32R)
    nc.vector.tensor_copy(out=ones1, in_=ones_f)

    bias_sb = []
    for i, o in enumerate([1024, 1024, 512, 512]):
        t = consts.tile([1, o], F32R, tag=f"b{i + 1}")
        nc.sync.dma_start(out=t, in_=io[f"b{i + 1}"])
        bias_sb.append(t)

    wa_sb = consts.tile([128, NHT * BPC * BPC], BF16)
    nc.sync.dma_start(out=wa_sb, in_=io["warep"])
    wa_m = wa_sb.rearrange("p (t b m) -> p t b m", t=NHT, b=BPC)

    madd_sb = consts.tile([BPC, S], F32)
    nc.sync.dma_start(out=madd_sb, in_=io["madd"])

    hT_sb = consts.tile([128, RNN // 128, BPC], F32R)
    nc.sync.dma_start(
        out=hT_sb, in_=io["hT"].rearrange("(u p) b -> p u b", p=128)
    )

    # ---- phase 1: MLP (float32r matmuls) ----
    def layer(xT_sb, K, O, wt_dram, bias_t, name):
        y_sb = mlp.tile([BPC, O], F32, tag=f"y_{name}")
        nch = O // 512
        pss = [
            psA.tile([BPC, 512], F32, tag="ps_small", name=f"ps_y{name}_{n}")
            for n in range(nch)
        ]
        for n in range(nch):
            nc.tensor.matmul(
                pss[n],
                lhsT=ones1,
                rhs=bias_t[0:1, n * 512 : (n + 1) * 512],
                start=True,
                stop=False,
            )
        kt = K // 128
        for k in range(kt):
            wt = wpool.tile([128, O], F32R, tag="wt")
            nc.sync.dma_start(out=wt, in_=wt_dram[k * 128 : (k + 1) * 128, :])
            for n in range(nch):
                nc.tensor.matmul(
                    pss[n],
                    lhsT=xT_sb[:, k, :],
                    rhs=wt[:, n * 512 : (n + 1) * 512],
                    start=False,
                    stop=(k == kt - 1),
                )
        for n in range(nch):
            nc.scalar.copy(out=y_sb[:, n * 512 : (n + 1) * 512], in_=pss[n])
        return y_sb

    def transpose_rows(y_sb, O, name, dtype=F32R):
        yT = mlp.tile([128, O // 128, BPC], dtype, tag=f"yT_{name}")
        for j in range(O // 128):
            ps = psA.tile([128, BPC], F32, tag="ps_small")
            nc.tensor.transpose(ps, y_sb[:, j * 128 : (j + 1) * 128], ident[:BPC, :BPC])
            nc.vector.tensor_copy(out=yT[:, j, :], in_=ps)
        return yT

    y1 = layer(hT_sb, RNN, 1024, io["w1t"], bias_sb[0], "1")
    y1T = transpose_rows(y1, 1024, "1")
    y2 = layer(y1T, 1024, 1024, io["w2t"], bias_sb[1], "2")
    y2T = transpose_rows(y2, 1024, "2")
    y3 = layer(y2T, 1024, 512, io["w3t"], bias_sb[2], "3")
    y3T = transpose_rows(y3, 512, "3")
    ah = layer(y3T, 512, 512, io["w4t"], bias_sb[3], "4")
    ahT = transpose_rows(ah, 512, "ah", dtype=F32)  # [128, NHT, BPC]

    w_mask = mlp.tile([128, NST, BPC, BPC], BF16, tag="w_mask")
    nc.vector.memset(w_mask, 0.0)

    # ---- phase 2: scores = Wa . tanh(p^T + att_h) + (mask_add + ba) ----
    # lhsT column m of wa_m[:, ht, b, :] is Wa if m == b else 0, so batch b's
    # matmuls only contribute to PSUM row b; all 16 batches (x NHT k-tiles)
    # accumulate into one [BPC, 512] PSUM group per s-half.
    scores = mlp.tile([BPC, S], F32, tag="scores")
    nsh = S // 512
    ps_sc = [
        psA.tile([BPC, 512], F32, tag="ps_small", name=f"ps_sc_{sh}")
        for sh in range(nsh)
    ]
    for b in range(BPC):
        pt = ppool.tile([128, NHT, S], BF16, tag="pt")
        nc.sync.dma_start(
            out=pt, in_=io["pT"][b].rearrange("(u p) s -> p u s", p=128)
        )
        for ht in range(NHT):
            nc.scalar.activation(
                out=pt[:, ht, :],
                in_=pt[:, ht, :],
                func=TANH,
                bias=ahT[:, ht, b : b + 1],
                scale=1.0,
            )
        for sh in range(nsh):
            for ht in range(NHT):
                nc.tensor.matmul(
                    ps_sc[sh],
                    lhsT=wa_m[:, ht, b, :],
                    rhs=pt[:, ht, sh * 512 : (sh + 1) * 512],
                    start=(b == 0 and ht == 0),
                    stop=(b == BPC - 1 and ht == NHT - 1),
                )
    for sh in range(nsh):
        nc.vector.tensor_add(
            out=scores[:, sh * 512 : (sh + 1) * 512],
            in0=ps_sc[sh],
            in1=madd_sb[:, sh * 512 : (sh + 1) * 512],
        )

    # ---- phase 2.5: softmax over S (in place on scores) ----
    mx = mlp.tile([BPC, 1], F32, tag="mx")
    nc.vector.reduce_max(out=mx, in_=scores, axis=AX_X)
    nmx = mlp.tile([BPC, 1], F32, tag="nmx")
    nc.vector.tensor_scalar_mul(out=nmx, in0=mx, scalar1=-1.0)
    ssum = mlp.tile([BPC, 1], F32, tag="ssum")
    nc.scalar.activation(
        out=scores, in_=scores, func=EXP, bias=nmx, scale=1.0, accum_out=ssum
    )
    rs = mlp.tile([BPC, 1], F32, tag="rs")
    nc.vector.reciprocal(out=rs, in_=ssum)

    # Block-diagonal masked weights: w_mask[:, t, b, m] = w[s, b] if m == b
    # else 0, so batch b's matvec only writes PSUM row b.  The PE-transposed
    # (unnormalized) softmax weights are written straight onto the diagonal;
    # 1/sum is folded into the final PSUM evacuation instead.
    for t in range(NST):
        ps = psA.tile([128, BPC], F32, tag="ps_small")
        nc.tensor.transpose(ps, scores[:, t * 128 : (t + 1) * 128], ident[:BPC, :BPC])
        sl = w_mask[:, t, :, :]
        diag_ap = bass.AP(
            tensor=sl.tensor,
            offset=sl.offset,
            ap=[sl.ap[0], [sl.ap[1][0] + sl.ap[2][0], BPC]],
        )
        nc.vector.tensor_copy(out=diag_ap, in_=ps)

    # ---- phase 3: att_res[b] = sum_s w[b,s] * att_feats[b,s,:] ----
    out_sb = mlp.tile([BPC, RNN], F32, tag="out_sb")
    nn = RNN // 512
    ps_mv = [
        psB.tile([BPC, 512], F32, tag="mv", name=f"ps_mv_{n}") for n in range(nn)
    ]
    for b in range(BPC):
        for tc_i in range(NST // FU):
            ft = fpool.tile([128, FU, RNN], BF16, tag="ft")
            eng = nc.sync if (b * (NST // FU) + tc_i) % 2 == 0 else nc.gpsimd
            eng.dma_start(
                out=ft,
                in_=io["f"][b, tc_i * FU * 128 : (tc_i + 1) * FU * 128, :].rearrange(
                    "(u p) d -> p u d", p=128
                ),
            )
            for u in range(FU):
                t = tc_i * FU + u
                for n in range(nn):
                    nc.tensor.matmul(
                        ps_mv[n],
                        lhsT=w_mask[:, t, b, :],
                        rhs=ft[:, u, n * 512 : (n + 1) * 512],
                        start=(b == 0 and t == 0),
                        stop=(b == BPC - 1 and t == NST - 1),
                    )
    for n in range(nn):
        nc.vector.tensor_scalar_mul(
            out=out_sb[:, n * 512 : (n + 1) * 512], in0=ps_mv[n], scalar1=rs
        )
    nc.sync.dma_start(out=io["out"], in_=out_sb)


def _build():
    from contextlib import ExitStack

    nc = bacc.Bacc("TRN2", target_bir_lowering=False, debug=False, num_devices=N_CORES)
    io = {
        "hT": nc.dram_tensor("hT", [RNN, BPC], F32R, kind="ExternalInput").ap(),
        "pT": nc.dram_tensor("pT", [BPC, HID, S], BF16, kind="ExternalInput").ap(),
        "f": nc.dram_tensor("f", [BPC, S, RNN], BF16, kind="ExternalInput").ap(),
        "madd": nc.dram_tensor("madd", [BPC, S], F32, kind="ExternalInput").ap(),
        "w1t": nc.dram_tensor("w1t", [RNN, 1024], F32R, kind="ExternalInput").ap(),
        "w2t": nc.dram_tensor("w2t", [1024, 1024], F32R, kind="ExternalInput").ap(),
        "w3t": nc.dram_tensor("w3t", [1024, 512], F32R, kind="ExternalInput").ap(),
        "w4t": nc.dram_tensor("w4t", [512, 512], F32R, kind="ExternalInput").ap(),
        "b1": nc.dram_tensor("b1", [1, 1024], F32R, kind="ExternalInput").ap(),
        "b2": nc.dram_tensor("b2", [1, 1024], F32R, kind="ExternalInput").ap(),
        "b3": nc.dram_tensor("b3", [1, 512], F32R, kind="ExternalInput").ap(),
        "b4": nc.dram_tensor("b4", [1, 512], F32R, kind="ExternalInput").ap(),
        "warep": nc.dram_tensor(
            "warep", [128, NHT * BPC * BPC], BF16, kind="ExternalInput"
        ).ap(),
        "out": nc.dram_tensor("out", [BPC, RNN], F32, kind="ExternalOutput").ap(),
    }
    with tile.TileContext(nc) as tc:
        with ExitStack() as ctx:
            _build_body(ctx, tc, io)
    nc.compile()
    return nc


@functools.lru_cache(maxsize=1)
def _get_nc():
    return _build()


def _prep_in_maps(h, att_feats, p_att_feats, mask, W1, b1, W2, b2, W3, b3, W4, b4, Wa, ba):
    f32 = np.float32
    asc = np.ascontiguousarray

    def a(x):
        return np.asarray(x, dtype=f32)

    w1t = asc(a(W1).T)
    w2t = asc(a(W2).T)
    w3t = asc(a(W3).T)
    w4t = asc(a(W4).T)
    b1r = a(b1).reshape(1, -1)
    b2r = a(b2).reshape(1, -1)
    b3r = a(b3).reshape(1, -1)
    b4r = a(b4).reshape(1, -1)
    wa = a(Wa).reshape(-1)  # [HID]
    warep = np.zeros((128, NHT, BPC, BPC), dtype=f32)
    for ht in range(NHT):
        for b in range(BPC):
            warep[:, ht, b, b] = wa[ht * 128 : (ht + 1) * 128]
    warep = warep.reshape(128, NHT * BPC * BPC).astype(ml_dtypes.bfloat16)
    ba0 = float(np.asarray(ba).reshape(-1)[0])

    h = a(h)
    p = np.asarray(p_att_feats, dtype=ml_dtypes.bfloat16)
    f = np.asarray(att_feats, dtype=ml_dtypes.bfloat16)
    m = np.asarray(mask)

    in_maps = []
    for c in range(N_CORES):
        sl = slice(c * BPC, (c + 1) * BPC)
        in_maps.append(
            {
                "hT": asc(h[sl].T),
                "pT": asc(p[sl].transpose(0, 2, 1)),
                "f": asc(f[sl]),
                "madd": (m[sl].astype(f32) * MASK_NEG + ba0).astype(f32),
                "w1t": w1t,
                "w2t": w2t,
                "w3t": w3t,
                "w4t": w4t,
                "b1": b1r,
                "b2": b2r,
                "b3": b3r,
                "b4": b4r,
                "warep": warep,
            }
        )
    return in_maps


def _run(in_maps, trace=False):
    nc = _get_nc()
    res = run_bass_kernel_spmd(nc, in_maps, core_ids=list(range(N_CORES)), trace=trace)
    out = np.concatenate([res.results[c]["out"] for c in range(N_CORES)], axis=0)
    return out, res


def kernel(h, att_feats, p_att_feats, mask, W1, b1, W2, b2, W3, b3, W4, b4, Wa, ba):
    in_maps = _prep_in_maps(
        h, att_feats, p_att_feats, mask, W1, b1, W2, b2, W3, b3, W4, b4, Wa, ba
    )
    out, _ = _run(in_maps)
    return out


# revision 12
# speedup vs baseline: 1.7567x; 1.0307x over previous
"""Trainium2 Bass kernel for the Attention3 module (B=128, S=1024, RNN=2048, HID=512).

Strategy: data-parallel over batch B across 8 NeuronCores (16 batches/core).
Host side only reshapes/transposes/downcasts inputs into DMA-friendly layouts;
all model compute (MLP, tanh, scores, softmax, weighted sum) runs on device.

Per-core device pipeline (batches processed in two half-groups of 8 so the
first half's weighted-sum streams att_feats while the second half's scores are
still being produced):
  1. MLP: att_h = h@W1.T+b1 @W2.T+b2 @W3.T+b3 @W4.T+b4   (PE, bf16 in / f32 acc)
     - activations kept transposed ([K,16] lhsT tiles); weights pre-transposed
       on host; biases folded in as K=1 ones-outer-product matmuls into the
       same PSUM accumulation group.
  2. scores: tanh(p_att^T + att_h) with HID on partitions, so the att_h add is
     a fused per-partition bias on ScalarE (in-place on the streamed p tile);
     Wa contraction is a PE matmul whose stationary operand column m holds Wa
     masked to batch b (zero elsewhere), so each batch of a half-group
     accumulates into its own PSUM row of one shared [8, 512] group per s-half.
     Mask+ba applied as a precomputed additive f32 term during evacuation.
  3. softmax over S per half-group on [8, 1024]; exp output (unnormalized) is
     PE-transposed straight onto the block-diagonal of the masked weight
     tensor; 1/sum is folded into the final PSUM evacuation.
  4. weighted sum: stream att_feats tiles [128, 2, 2048] (bf16) and matmul;
     each batch lands in its own row of shared [8, 512] PSUM groups.

DMA: bulk streams are >= 1 MiB and split between the SP HWDGE ring (nc.sync)
and the SWDGE path (nc.gpsimd) so two transfers stay in flight.
"""

import functools

import ml_dtypes
import numpy as np

import concourse.bacc as bacc
import concourse.bass as bass
import concourse.tile as tile
from concourse import mybir
from concourse.bass_utils import run_bass_kernel_spmd
from concourse.masks import make_identity

N_CORES = 8
B, S, RNN, HID = 128, 1024, 2048, 512
BPC = B // N_CORES  # batches per core
HB = BPC // 2  # half-group size
F32 = mybir.dt.float32
BF16 = mybir.dt.bfloat16
MASK_NEG = -1.0e9
AX_X = mybir.AxisListType.X
TANH = mybir.ActivationFunctionType.Tanh
EXP = mybir.ActivationFunctionType.Exp

NHT = HID // 128  # 4 h-tiles
NST = S // 128  # 8 s-tiles
FU = 2  # s-tiles per att_feats DMA
NN = RNN // 512  # 4 output chunks
NSH = S // 512  # 2 score halves


def _build_body(ctx, tc, io):
    nc = tc.nc

    consts = ctx.enter_context(tc.tile_pool(name="consts", bufs=1))
    wpool = ctx.enter_context(tc.tile_pool(name="wpool", bufs=3))
    mlp = ctx.enter_context(tc.tile_pool(name="mlp", bufs=1))
    ppool = ctx.enter_context(tc.tile_pool(name="ppool", bufs=6))
    fpool = ctx.enter_context(tc.tile_pool(name="fpool", bufs=9))
    psA = ctx.enter_context(tc.tile_pool(name="psA", bufs=3, space="PSUM"))
    psB = ctx.enter_context(tc.tile_pool(name="psB", bufs=4, space="PSUM"))

    # ---- constants / small inputs ----
    ident = consts.tile([128, 128], F32)
    make_identity(nc, ident)
    ones_f = consts.tile([1, BPC], F32)
    nc.vector.memset(ones_f, 1.0)
    ones1 = consts.tile([1, BPC], BF16)
    nc.vector.tensor_copy(out=ones1, in_=ones_f)

    bias_sb = []
    for i, o in enumerate([1024, 1024, 512, 512]):
        t = consts.tile([1, o], BF16, tag=f"b{i + 1}")
        nc.sync.dma_start(out=t, in_=io[f"b{i + 1}"])
        bias_sb.append(t)

    wa_sb = consts.tile([128, NHT * BPC * BPC], BF16)
    nc.sync.dma_start(out=wa_sb, in_=io["warep"])
    wa_m = wa_sb.rearrange("p (t b m) -> p t b m", t=NHT, b=BPC)

    madd_sb = consts.tile([HB, 2, S], F32)
    nc.sync.dma_start(out=madd_sb, in_=io["madd"])

    hT_sb = consts.tile([128, RNN // 128, BPC], BF16)
    nc.sync.dma_start(out=hT_sb, in_=io["hT"].rearrange("(u p) b -> p u b", p=128))

    # ---- phase 1: MLP (bf16 matmuls, f32 accumulate) ----
    def layer(xT_sb, K, O, wt_dram, bias_t, name):
        y_sb = mlp.tile([BPC, O], F32, tag=f"y_{name}")
        nch = O // 512
        pss = [
            psA.tile([BPC, 512], F32, tag="ps_small", name=f"ps_y{name}_{n}")
            for n in range(nch)
        ]
        for n in range(nch):
            nc.tensor.matmul(
                pss[n],
                lhsT=ones1,
                rhs=bias_t[0:1, n * 512 : (n + 1) * 512],
                start=True,
                stop=False,
            )
        kt = K // 128
        for k2 in range(kt // 2):
            wt = wpool.tile([128, 2, O], BF16, tag="wt")
            nc.sync.dma_start(
                out=wt,
                in_=wt_dram[k2 * 256 : (k2 + 1) * 256, :].rearrange(
                    "(u p) o -> p u o", p=128
                ),
            )
            for u in range(2):
                k = k2 * 2 + u
                for n in range(nch):
                    nc.tensor.matmul(
                        pss[n],
                        lhsT=xT_sb[:, k, :],
                        rhs=wt[:, u, n * 512 : (n + 1) * 512],
                        start=False,
                        stop=(k == kt - 1),
                    )
        for n in range(nch):
            nc.scalar.copy(out=y_sb[:, n * 512 : (n + 1) * 512], in_=pss[n])
        return y_sb

    def transpose_rows(y_sb, O, name, dtype=BF16):
        yT = mlp.tile([128, O // 128, BPC], dtype, tag=f"yT_{name}")
        for j in range(O // 128):
            ps = psA.tile([128, BPC], F32, tag="ps_small")
            nc.tensor.transpose(ps, y_sb[:, j * 128 : (j + 1) * 128], ident[:BPC, :BPC])
            nc.vector.tensor_copy(out=yT[:, j, :], in_=ps)
        return yT

    y1 = layer(hT_sb, RNN, 1024, io["w1t"], bias_sb[0], "1")
    y1T = transpose_rows(y1, 1024, "1")
    y2 = layer(y1T, 1024, 1024, io["w2t"], bias_sb[1], "2")
    y2T = transpose_rows(y2, 1024, "2")
    y3 = layer(y2T, 1024, 512, io["w3t"], bias_sb[2], "3")
    y3T = transpose_rows(y3, 512, "3")
    ah = layer(y3T, 512, 512, io["w4t"], bias_sb[3], "4")
    ahT = transpose_rows(ah, 512, "ah", dtype=F32)  # [128, NHT, BPC]

    # Block-diagonal masked softmax weights (zeroed early, off the critical
    # path): w_mask[:, t, b, m] = exp_w[s, b] if m == b else 0, so batch b's
    # matvec only writes its own PSUM row within its half-group.
    w_mask = mlp.tile([128, NST, BPC, BPC], BF16, tag="w_mask")
    nc.vector.memset(w_mask, 0.0)

    rs_g = []

    def scores_half(g):
        """Scores + softmax + masked-weight diagonal for batches g*8..g*8+7."""
        scores = mlp.tile([HB, S], F32, tag=f"scores{g}")
        ps_sc = [
            psA.tile([HB, 512], F32, tag="ps_small", name=f"ps_sc_{g}_{sh}")
            for sh in range(NSH)
        ]
        for bl in range(HB):
            b = g * HB + bl
            pt = ppool.tile([128, NHT, S], BF16, tag="pt")
            nc.sync.dma_start(
                out=pt, in_=io["pT"][b].rearrange("(u p) s -> p u s", p=128)
            )
            for ht in range(NHT):
                nc.scalar.activation(
                    out=pt[:, ht, :],
                    in_=pt[:, ht, :],
                    func=TANH,
                    bias=ahT[:, ht, b : b + 1],
                    scale=1.0,
                )
            for sh in range(NSH):
                for ht in range(NHT):
                    nc.tensor.matmul(
                        ps_sc[sh],
                        lhsT=wa_m[:, ht, b, g * HB : (g + 1) * HB],
                        rhs=pt[:, ht, sh * 512 : (sh + 1) * 512],
                        start=(bl == 0 and ht == 0),
                        stop=(bl == HB - 1 and ht == NHT - 1),
                    )
        for sh in range(NSH):
            nc.vector.tensor_add(
                out=scores[:, sh * 512 : (sh + 1) * 512],
                in0=ps_sc[sh],
                in1=madd_sb[:, g, sh * 512 : (sh + 1) * 512],
            )
        mx = mlp.tile([HB, 1], F32, tag=f"mx{g}")
        nc.vector.reduce_max(out=mx, in_=scores, axis=AX_X)
        nmx = mlp.tile([HB, 1], F32, tag=f"nmx{g}")
        nc.vector.tensor_scalar_mul(out=nmx, in0=mx, scalar1=-1.0)
        ssum = mlp.tile([HB, 1], F32, tag=f"ssum{g}")
        nc.scalar.activation(
            out=scores, in_=scores, func=EXP, bias=nmx, scale=1.0, accum_out=ssum
        )
        rs = mlp.tile([HB, 1], F32, tag=f"rs{g}")
        nc.vector.reciprocal(out=rs, in_=ssum)
        rs_g.append(rs)
        for t in range(NST):
            ps = psA.tile([128, HB], F32, tag="ps_small")
            nc.tensor.transpose(
                ps, scores[:, t * 128 : (t + 1) * 128], ident[:HB, :HB]
            )
            sl = w_mask[:, t, :, :]
            diag_ap = bass.AP(
                tensor=sl.tensor,
                offset=sl.offset + g * HB * (BPC + 1),
                ap=[sl.ap[0], [BPC + 1, HB]],
            )
            nc.vector.tensor_copy(out=diag_ap, in_=ps)

    def matvec_half(g):
        """att_res[b] = (sum_s exp_w[b,s] * att_feats[b,s,:]) / sum for the half."""
        out_sb = mlp.tile([HB, RNN], F32, tag=f"out_sb{g}")
        ps_mv = [
            psB.tile([HB, 512], F32, tag="mv", name=f"ps_mv_{g}_{n}")
            for n in range(NN)
        ]
        for bl in range(HB):
            b = g * HB + bl
            for tc_i in range(NST // FU):
                ft = fpool.tile([128, FU, RNN], BF16, tag="ft")
                eng = nc.sync if (bl * (NST // FU) + tc_i) % 2 == 0 else nc.gpsimd
                eng.dma_start(
                    out=ft,
                    in_=io["f"][
                        b, tc_i * FU * 128 : (tc_i + 1) * FU * 128, :
                    ].rearrange("(u p) d -> p u d", p=128),
                )
                for u in range(FU):
                    t = tc_i * FU + u
                    for n in range(NN):
                        nc.tensor.matmul(
                            ps_mv[n],
                            lhsT=w_mask[:, t, b, g * HB : (g + 1) * HB],
                            rhs=ft[:, u, n * 512 : (n + 1) * 512],
                            start=(bl == 0 and t == 0),
                            stop=(bl == HB - 1 and t == NST - 1),
                        )
        for n in range(NN):
            nc.vector.tensor_scalar_mul(
                out=out_sb[:, n * 512 : (n + 1) * 512], in0=ps_mv[n], scalar1=rs_g[g]
            )
        nc.sync.dma_start(out=io["out"][g * HB : (g + 1) * HB, :], in_=out_sb)

    scores_half(0)
    matvec_half(0)
    scores_half(1)
    matvec_half(1)


def _build():
    from contextlib import ExitStack

    nc = bacc.Bacc("TRN2", target_bir_lowering=False, debug=False, num_devices=N_CORES)
    io = {
        "hT": nc.dram_tensor("hT", [RNN, BPC], BF16, kind="ExternalInput").ap(),
        "pT": nc.dram_tensor("pT", [BPC, HID, S], BF16, kind="ExternalInput").ap(),
        "f": nc.dram_tensor("f", [BPC, S, RNN], BF16, kind="ExternalInput").ap(),
        "madd": nc.dram_tensor("madd", [HB, 2, S], F32, kind="ExternalInput").ap(),
        "w1t": nc.dram_tensor("w1t", [RNN, 1024], BF16, kind="ExternalInput").ap(),
        "w2t": nc.dram_tensor("w2t", [1024, 1024], BF16, kind="ExternalInput").ap(),
        "w3t": nc.dram_tensor("w3t", [1024, 512], BF16, kind="ExternalInput").ap(),
        "w4t": nc.dram_tensor("w4t", [512, 512], BF16, kind="ExternalInput").ap(),
        "b1": nc.dram_tensor("b1", [1, 1024], BF16, kind="ExternalInput").ap(),
        "b2": nc.dram_tensor("b2", [1, 1024], BF16, kind="ExternalInput").ap(),
        "b3": nc.dram_tensor("b3", [1, 512], BF16, kind="ExternalInput").ap(),
        "b4": nc.dram_tensor("b4", [1, 512], BF16, kind="ExternalInput").ap(),
        "warep": nc.dram_tensor(
            "warep", [128, NHT * BPC * BPC], BF16, kind="ExternalInput"
        ).ap(),
        "out": nc.dram_tensor("out", [BPC, RNN], F32, kind="ExternalOutput").ap(),
    }
    with tile.TileContext(nc) as tc:
        with ExitStack() as ctx:
            _build_body(ctx, tc, io)
    nc.compile()
    return nc


@functools.lru_cache(maxsize=1)
def _get_nc():
    return _build()


def _prep_in_maps(h, att_feats, p_att_feats, mask, W1, b1, W2, b2, W3, b3, W4, b4, Wa, ba):
    f32 = np.float32
    bf16 = ml_dtypes.bfloat16
    asc = np.ascontiguousarray

    def abf(x):
        return np.asarray(x).astype(bf16)

    w1t = asc(np.asarray(W1, dtype=f32).T).astype(bf16)
    w2t = asc(np.asarray(W2, dtype=f32).T).astype(bf16)
    w3t = asc(np.asarray(W3, dtype=f32).T).astype(bf16)
    w4t = asc(np.asarray(W4, dtype=f32).T).astype(bf16)
    b1r = abf(b1).reshape(1, -1)
    b2r = abf(b2).reshape(1, -1)
    b3r = abf(b3).reshape(1, -1)
    b4r = abf(b4).reshape(1, -1)
    wa = np.asarray(Wa, dtype=f32).reshape(-1)  # [HID]
    warep = np.zeros((128, NHT, BPC, BPC), dtype=f32)
    for ht in range(NHT):
        for b in range(BPC):
            warep[:, ht, b, b] = wa[ht * 128 : (ht + 1) * 128]
    warep = warep.reshape(128, NHT * BPC * BPC).astype(bf16)
    ba0 = float(np.asarray(ba).reshape(-1)[0])

    h = np.asarray(h, dtype=f32)
    p = np.asarray(p_att_feats).astype(bf16)
    f = np.asarray(att_feats).astype(bf16)
    m = np.asarray(mask)

    in_maps = []
    for c in range(N_CORES):
        sl = slice(c * BPC, (c + 1) * BPC)
        madd = (m[sl].astype(f32) * MASK_NEG + ba0).astype(f32)
        in_maps.append(
            {
                "hT": asc(h[sl].T).astype(bf16),
                "pT": asc(p[sl].transpose(0, 2, 1)),
                "f": asc(f[sl]),
                "madd": asc(madd.reshape(2, HB, S).transpose(1, 0, 2)),
                "w1t": w1t,
                "w2t": w2t,
                "w3t": w3t,
                "w4t": w4t,
                "b1": b1r,
                "b2": b2r,
                "b3": b3r,
                "b4": b4r,
                "warep": warep,
            }
        )
    return in_maps


def _run(in_maps, trace=False):
    nc = _get_nc()
    res = run_bass_kernel_spmd(nc, in_maps, core_ids=list(range(N_CORES)), trace=trace)
    out = np.concatenate([res.results[c]["out"] for c in range(N_CORES)], axis=0)
    return out, res


def kernel(h, att_feats, p_att_feats, mask, W1, b1, W2, b2, W3, b3, W4, b4, Wa, ba):
    in_maps = _prep_in_maps(
        h, att_feats, p_att_feats, mask, W1, b1, W2, b2, W3, b3, W4, b4, Wa, ba
    )
    out, _ = _run(in_maps)
    return out


# revision 13
# speedup vs baseline: 1.8987x; 1.0808x over previous
"""Trainium2 Bass kernel for the Attention3 module (B=128, S=1024, RNN=2048, HID=512).

Strategy: data-parallel over batch B across 8 NeuronCores (16 batches/core).
Host side only reshapes/transposes/downcasts inputs into DMA-friendly layouts;
all model compute (MLP, tanh, scores, softmax, weighted sum) runs on device.

Per-core device pipeline (batches processed in two half-groups of 8 so the
first half's weighted-sum streams att_feats while the second half's scores are
still being produced):
  1. MLP: att_h = h@W1.T+b1 @W2.T+b2 @W3.T+b3 @W4.T+b4   (PE, bf16 in / f32 acc)
     - activations kept transposed ([K,16] lhsT tiles); weights pre-transposed
       on host; biases folded in as K=1 ones-outer-product matmuls into the
       same PSUM accumulation group.
  2. scores: tanh(p_att^T + att_h) with HID on partitions, so the att_h add is
     a fused per-partition bias on ScalarE (in-place on the streamed p tile);
     Wa contraction is a PE matmul whose stationary operand column m holds Wa
     masked to batch b (zero elsewhere), so each batch of a half-group
     accumulates into its own PSUM row of one shared [8, 512] group per s-half.
     Mask+ba applied as a precomputed additive f32 term during evacuation.
  3. softmax over S per half-group on [8, 1024]; exp output (unnormalized) is
     PE-transposed straight onto the block-diagonal of the masked weight
     tensor; 1/sum is folded into the final PSUM evacuation.
  4. weighted sum: stream att_feats tiles [128, 2, 2048] (bf16) and matmul;
     each batch lands in its own row of shared [8, 512] PSUM groups.

DMA: bulk streams are >= 1 MiB and split between the SP HWDGE ring (nc.sync)
and the SWDGE path (nc.gpsimd) so two transfers stay in flight.
"""

import functools

import ml_dtypes
import numpy as np

import concourse.bacc as bacc
import concourse.bass as bass
import concourse.tile as tile
from concourse import mybir
from concourse.bass_utils import run_bass_kernel_spmd
from concourse.masks import make_identity

N_CORES = 8
B, S, RNN, HID = 128, 1024, 2048, 512
BPC = B // N_CORES  # batches per core
HB = BPC // 2  # half-group size
F32 = mybir.dt.float32
BF16 = mybir.dt.bfloat16
MASK_NEG = -1.0e9
AX_X = mybir.AxisListType.X
TANH = mybir.ActivationFunctionType.Tanh
EXP = mybir.ActivationFunctionType.Exp

NHT = HID // 128  # 4 h-tiles
NST = S // 128  # 8 s-tiles
FU = 2  # s-tiles per att_feats DMA
NN = RNN // 512  # 4 output chunks
NSH = S // 512  # 2 score halves


def _build_body(ctx, tc, io):
    nc = tc.nc

    consts = ctx.enter_context(tc.tile_pool(name="consts", bufs=1))
    wpool = ctx.enter_context(tc.tile_pool(name="wpool", bufs=3))
    mlp = ctx.enter_context(tc.tile_pool(name="mlp", bufs=1))
    ppool = ctx.enter_context(tc.tile_pool(name="ppool", bufs=6))
    fpool = ctx.enter_context(tc.tile_pool(name="fpool", bufs=9))
    psA = ctx.enter_context(tc.tile_pool(name="psA", bufs=3, space="PSUM"))
    psB = ctx.enter_context(tc.tile_pool(name="psB", bufs=4, space="PSUM"))

    # ---- constants / small inputs ----
    ident = consts.tile([128, 128], F32)
    make_identity(nc, ident)
    ones_f = consts.tile([1, BPC], F32)
    nc.vector.memset(ones_f, 1.0)
    ones1 = consts.tile([1, BPC], BF16)
    nc.vector.tensor_copy(out=ones1, in_=ones_f)

    bias_sb = []
    for i, o in enumerate([1024, 1024, 512, 512]):
        t = consts.tile([1, o], BF16, tag=f"b{i + 1}")
        nc.sync.dma_start(out=t, in_=io[f"b{i + 1}"])
        bias_sb.append(t)

    wa_sb = consts.tile([128, NHT * BPC * BPC], BF16)
    nc.sync.dma_start(out=wa_sb, in_=io["warep"])
    wa_m = wa_sb.rearrange("p (t b m) -> p t b m", t=NHT, b=BPC)

    madd_sb = consts.tile([HB, 2, S], F32)
    nc.sync.dma_start(out=madd_sb, in_=io["madd"])

    hT_sb = consts.tile([128, RNN // 128, BPC], BF16)
    nc.sync.dma_start(out=hT_sb, in_=io["hT"].rearrange("(u p) b -> p u b", p=128))

    # ---- phase 1: MLP (bf16 matmuls, f32 accumulate) ----
    def layer(xT_sb, K, O, wt_dram, bias_t, name):
        y_sb = mlp.tile([BPC, O], F32, tag=f"y_{name}")
        nch = O // 512
        pss = [
            psA.tile([BPC, 512], F32, tag="ps_small", name=f"ps_y{name}_{n}")
            for n in range(nch)
        ]
        for n in range(nch):
            nc.tensor.matmul(
                pss[n],
                lhsT=ones1,
                rhs=bias_t[0:1, n * 512 : (n + 1) * 512],
                start=True,
                stop=False,
            )
        kt = K // 128
        for k2 in range(kt // 2):
            wt = wpool.tile([128, 2, O], BF16, tag="wt")
            nc.sync.dma_start(
                out=wt,
                in_=wt_dram[k2 * 256 : (k2 + 1) * 256, :].rearrange(
                    "(u p) o -> p u o", p=128
                ),
            )
            for u in range(2):
                k = k2 * 2 + u
                for n in range(nch):
                    nc.tensor.matmul(
                        pss[n],
                        lhsT=xT_sb[:, k, :],
                        rhs=wt[:, u, n * 512 : (n + 1) * 512],
                        start=False,
                        stop=(k == kt - 1),
                    )
        for n in range(nch):
            nc.scalar.copy(out=y_sb[:, n * 512 : (n + 1) * 512], in_=pss[n])
        return y_sb

    def transpose_rows(y_sb, O, name, dtype=BF16):
        yT = mlp.tile([128, O // 128, BPC], dtype, tag=f"yT_{name}")
        for j in range(O // 128):
            ps = psA.tile([128, BPC], F32, tag="ps_small")
            nc.tensor.transpose(ps, y_sb[:, j * 128 : (j + 1) * 128], ident[:BPC, :BPC])
            nc.vector.tensor_copy(out=yT[:, j, :], in_=ps)
        return yT

    y1 = layer(hT_sb, RNN, 1024, io["w1t"], bias_sb[0], "1")
    y1T = transpose_rows(y1, 1024, "1")
    y2 = layer(y1T, 1024, 1024, io["w2t"], bias_sb[1], "2")
    y2T = transpose_rows(y2, 1024, "2")
    y3 = layer(y2T, 1024, 512, io["w3t"], bias_sb[2], "3")
    y3T = transpose_rows(y3, 512, "3")
    ah = layer(y3T, 512, 512, io["w4t"], bias_sb[3], "4")
    ahT = transpose_rows(ah, 512, "ah", dtype=F32)  # [128, NHT, BPC]

    # Block-diagonal masked softmax weights (zeroed early, off the critical
    # path): w_mask[:, t, b, m] = exp_w[s, b] if m == b else 0, so batch b's
    # matvec only writes its own PSUM row within its half-group.
    w_mask = mlp.tile([128, NST, BPC, BPC], BF16, tag="w_mask")
    nc.vector.memset(w_mask, 0.0)

    # Per-half state for the batch-interleaved pipeline below.
    sc_state = {}
    mv_state = {}
    rs_g = {}

    def emit_scores_batch(g, bl):
        """pt DMA + tanh + score matmuls for batch g*8+bl."""
        if g not in sc_state:
            sc_state[g] = [
                psA.tile([HB, 512], F32, tag="ps_small", name=f"ps_sc_{g}_{sh}")
                for sh in range(NSH)
            ]
        ps_sc = sc_state[g]
        b = g * HB + bl
        pt = ppool.tile([128, NHT, S], BF16, tag="pt", name=f"pt_{b}")
        # Half B's p tiles ride the ACT HWDGE ring, which is past its half-A
        # work by then; half A uses the SP ring.
        eng = nc.sync if g == 0 else nc.scalar
        eng.dma_start(out=pt, in_=io["pT"][b].rearrange("(u p) s -> p u s", p=128))
        for ht in range(NHT):
            nc.scalar.activation(
                out=pt[:, ht, :],
                in_=pt[:, ht, :],
                func=TANH,
                bias=ahT[:, ht, b : b + 1],
                scale=1.0,
            )
        for sh in range(NSH):
            for ht in range(NHT):
                nc.tensor.matmul(
                    ps_sc[sh],
                    lhsT=wa_m[:, ht, b, g * HB : (g + 1) * HB],
                    rhs=pt[:, ht, sh * 512 : (sh + 1) * 512],
                    start=(bl == 0 and ht == 0),
                    stop=(bl == HB - 1 and ht == NHT - 1),
                )

    def finish_scores(g):
        """Evacuate score PSUM, softmax, write masked-weight diagonal."""
        ps_sc = sc_state[g]
        scores = mlp.tile([HB, S], F32, tag=f"scores{g}", name=f"scores{g}")
        for sh in range(NSH):
            nc.vector.tensor_add(
                out=scores[:, sh * 512 : (sh + 1) * 512],
                in0=ps_sc[sh],
                in1=madd_sb[:, g, sh * 512 : (sh + 1) * 512],
            )
        mx = mlp.tile([HB, 1], F32, tag=f"mx{g}", name=f"mx{g}")
        nc.vector.reduce_max(out=mx, in_=scores, axis=AX_X)
        nmx = mlp.tile([HB, 1], F32, tag=f"nmx{g}", name=f"nmx{g}")
        nc.vector.tensor_scalar_mul(out=nmx, in0=mx, scalar1=-1.0)
        ssum = mlp.tile([HB, 1], F32, tag=f"ssum{g}", name=f"ssum{g}")
        nc.scalar.activation(
            out=scores, in_=scores, func=EXP, bias=nmx, scale=1.0, accum_out=ssum
        )
        rs = mlp.tile([HB, 1], F32, tag=f"rs{g}", name=f"rs{g}")
        nc.vector.reciprocal(out=rs, in_=ssum)
        rs_g[g] = rs
        for t in range(NST):
            ps = psA.tile([128, HB], F32, tag="ps_small", name=f"ps_tr{g}_{t}")
            nc.tensor.transpose(ps, scores[:, t * 128 : (t + 1) * 128], ident[:HB, :HB])
            sl = w_mask[:, t, :, :]
            diag_ap = bass.AP(
                tensor=sl.tensor,
                offset=sl.offset + g * HB * (BPC + 1),
                ap=[sl.ap[0], [BPC + 1, HB]],
            )
            nc.vector.tensor_copy(out=diag_ap, in_=ps)

    def emit_matvec_batch(g, bl):
        """ft DMA + weighted-sum matmuls for batch g*8+bl."""
        if g not in mv_state:
            mv_state[g] = [
                psB.tile([HB, 512], F32, tag="mv", name=f"ps_mv_{g}_{n}")
                for n in range(NN)
            ]
        ps_mv = mv_state[g]
        b = g * HB + bl
        for tc_i in range(NST // FU):
            ft = fpool.tile([128, FU, RNN], BF16, tag="ft", name=f"ft_{b}_{tc_i}")
            eng = nc.sync if (bl * (NST // FU) + tc_i) % 2 == 0 else nc.gpsimd
            eng.dma_start(
                out=ft,
                in_=io["f"][
                    b, tc_i * FU * 128 : (tc_i + 1) * FU * 128, :
                ].rearrange("(u p) d -> p u d", p=128),
            )
            for u in range(FU):
                t = tc_i * FU + u
                for n in range(NN):
                    nc.tensor.matmul(
                        ps_mv[n],
                        lhsT=w_mask[:, t, b, g * HB : (g + 1) * HB],
                        rhs=ft[:, u, n * 512 : (n + 1) * 512],
                        start=(bl == 0 and t == 0),
                        stop=(bl == HB - 1 and t == NST - 1),
                    )

    def finish_matvec(g):
        """Scale by 1/sum during PSUM evacuation and store the half."""
        ps_mv = mv_state[g]
        out_sb = mlp.tile([HB, RNN], F32, tag=f"out_sb{g}", name=f"out_sb{g}")
        for n in range(NN):
            nc.vector.tensor_scalar_mul(
                out=out_sb[:, n * 512 : (n + 1) * 512], in0=ps_mv[n], scalar1=rs_g[g]
            )
        nc.sync.dma_start(out=io["out"][g * HB : (g + 1) * HB, :], in_=out_sb)

    for bl in range(HB):
        emit_scores_batch(0, bl)
    finish_scores(0)
    for bl in range(HB):
        emit_scores_batch(1, bl)
        emit_matvec_batch(0, bl)
    finish_matvec(0)
    finish_scores(1)
    for bl in range(HB):
        emit_matvec_batch(1, bl)
    finish_matvec(1)


def _build():
    from contextlib import ExitStack

    nc = bacc.Bacc("TRN2", target_bir_lowering=False, debug=False, num_devices=N_CORES)
    io = {
        "hT": nc.dram_tensor("hT", [RNN, BPC], BF16, kind="ExternalInput").ap(),
        "pT": nc.dram_tensor("pT", [BPC, HID, S], BF16, kind="ExternalInput").ap(),
        "f": nc.dram_tensor("f", [BPC, S, RNN], BF16, kind="ExternalInput").ap(),
        "madd": nc.dram_tensor("madd", [HB, 2, S], F32, kind="ExternalInput").ap(),
        "w1t": nc.dram_tensor("w1t", [RNN, 1024], BF16, kind="ExternalInput").ap(),
        "w2t": nc.dram_tensor("w2t", [1024, 1024], BF16, kind="ExternalInput").ap(),
        "w3t": nc.dram_tensor("w3t", [1024, 512], BF16, kind="ExternalInput").ap(),
        "w4t": nc.dram_tensor("w4t", [512, 512], BF16, kind="ExternalInput").ap(),
        "b1": nc.dram_tensor("b1", [1, 1024], BF16, kind="ExternalInput").ap(),
        "b2": nc.dram_tensor("b2", [1, 1024], BF16, kind="ExternalInput").ap(),
        "b3": nc.dram_tensor("b3", [1, 512], BF16, kind="ExternalInput").ap(),
        "b4": nc.dram_tensor("b4", [1, 512], BF16, kind="ExternalInput").ap(),
        "warep": nc.dram_tensor(
            "warep", [128, NHT * BPC * BPC], BF16, kind="ExternalInput"
        ).ap(),
        "out": nc.dram_tensor("out", [BPC, RNN], F32, kind="ExternalOutput").ap(),
    }
    with tile.TileContext(nc) as tc:
        with ExitStack() as ctx:
            _build_body(ctx, tc, io)
    nc.compile()
    return nc


@functools.lru_cache(maxsize=1)
def _get_nc():
    return _build()


def _prep_in_maps(h, att_feats, p_att_feats, mask, W1, b1, W2, b2, W3, b3, W4, b4, Wa, ba):
    f32 = np.float32
    bf16 = ml_dtypes.bfloat16
    asc = np.ascontiguousarray

    def abf(x):
        return np.asarray(x).astype(bf16)

    w1t = asc(np.asarray(W1, dtype=f32).T).astype(bf16)
    w2t = asc(np.asarray(W2, dtype=f32).T).astype(bf16)
    w3t = asc(np.asarray(W3, dtype=f32).T).astype(bf16)
    w4t = asc(np.asarray(W4, dtype=f32).T).astype(bf16)
    b1r = abf(b1).reshape(1, -1)
    b2r = abf(b2).reshape(1, -1)
    b3r = abf(b3).reshape(1, -1)
    b4r = abf(b4).reshape(1, -1)
    wa = np.asarray(Wa, dtype=f32).reshape(-1)  # [HID]
    warep = np.zeros((128, NHT, BPC, BPC), dtype=f32)
    for ht in range(NHT):
        for b in range(BPC):
            warep[:, ht, b, b] = wa[ht * 128 : (ht + 1) * 128]
    warep = warep.reshape(128, NHT * BPC * BPC).astype(bf16)
    ba0 = float(np.asarray(ba).reshape(-1)[0])

    h = np.asarray(h, dtype=f32)
    p = np.asarray(p_att_feats).astype(bf16)
    f = np.asarray(att_feats).astype(bf16)
    m = np.asarray(mask)

    in_maps = []
    for c in range(N_CORES):
        sl = slice(c * BPC, (c + 1) * BPC)
        madd = (m[sl].astype(f32) * MASK_NEG + ba0).astype(f32)
        in_maps.append(
            {
                "hT": asc(h[sl].T).astype(bf16),
                "pT": asc(p[sl].transpose(0, 2, 1)),
                "f": asc(f[sl]),
                "madd": asc(madd.reshape(2, HB, S).transpose(1, 0, 2)),
                "w1t": w1t,
                "w2t": w2t,
                "w3t": w3t,
                "w4t": w4t,
                "b1": b1r,
                "b2": b2r,
                "b3": b3r,
                "b4": b4r,
                "warep": warep,
            }
        )
    return in_maps


def _run(in_maps, trace=False):
    nc = _get_nc()
    res = run_bass_kernel_spmd(nc, in_maps, core_ids=list(range(N_CORES)), trace=trace)
    out = np.concatenate([res.results[c]["out"] for c in range(N_CORES)], axis=0)
    return out, res


def kernel(h, att_feats, p_att_feats, mask, W1, b1, W2, b2, W3, b3, W4, b4, Wa, ba):
    in_maps = _prep_in_maps(
        h, att_feats, p_att_feats, mask, W1, b1, W2, b2, W3, b3, W4, b4, Wa, ba
    )
    out, _ = _run(in_maps)
    return out
